# revision 10
# baseline (speedup 1.0000x reference)
"""Trainium2 Bass kernel for nn_Attention2 (dense transformer block with
softmax over the heads axis).

Computation per (n, t) batch b (B = n*t = 4096 total, X_b = x[n,:,t,:].T is
[vv=25, c=512]):
    qkv = X_b @ w_qkv.T, split into q,k,v heads [h=8, 25, hd=64]
    s[h,i,j] = (q[h,i,:] . k[h,j,:]) / 8      (scale folded into w_q on host)
    p = softmax over h (axis 0)
    o[h,i,:] = sum_j p[h,i,j] v[h,j,:]  -> [25, 512] -> @ w_proj.T
    out[n,:,t,:] = result.T

Sharding: data-parallel over n, 2 n-values (512 batches) per core, 8 cores.

Layout trick: x[n, :, t, :] is naturally X_b^T ([c, vv], c on partitions), so
the qkv and proj matmuls run as weight-stationary batched GEMMs with the
batch dim fused into the moving free dim (16 batches -> N=400).  v is
computed in V[j, c'] layout (j on partitions) via x-slab-stationary matmuls
so the attention-core matmuls need no transposes.  fp32r (1 cycle/row vs 4
for fp32, ~1.6e-4 rel err) is used for the three big GEMMs; the tiny
attention matmuls run fp32 packed onto the PE array with tile_position.
"""
import numpy as np
import concourse.bass as bass
import concourse.mybir as mybir
import concourse.tile as tile
from concourse.bass_utils import run_bass_kernel_spmd
from concourse.vector_clock import ScopedClock, VectorClock

F32 = mybir.dt.float32
F32R = mybir.dt.float32r

N_CORES = 8
NN_PER_CORE = 2        # n values per core
T = 256
VV = 25
C = 512
H = 8
HD = 64
TG = 16                # t values (batches) per group
NGROUPS = NN_PER_CORE * (T // TG)   # 32 groups per core
NB = TG * VV           # 400 moving columns per group


def _split_drain_and_barrier(self, tick_clock, wait_clock):
    # walrus caps sync-wait commands at 1 for CTRL_NO; split the kernel-tail
    # drain into one drain per pending proc.
    vc = tick_clock.global_clock
    n = len(vc)
    for i in range(n):
        if vc[i] == 0:
            continue
        sub = VectorClock([vc[j] if j == i else 0 for j in range(n)])
        d = self.nc.sync.drain()
        wait_clock.add_sem_waits(d.ins, ScopedClock({None: sub}))
    self.nc.all_engine_barrier()
    assert self.sems is not None
    popped = self.nc._tile_sem_poison_stack.pop()
    assert popped is self._sem_poison
    self.nc.clear_and_free_semaphores(list(self.sems.allocated().values()))
    self.nc.all_engine_barrier()


tile.TileContext._drain_and_barrier = _split_drain_and_barrier


def split_excess_waits(nc, limit=1):
    """walrus codegen allows very few sync-wait commands per instruction
    (1 for matmul/drain/DMA structs).  Move excess waits onto same-engine
    NoOp carriers inserted just before the instruction — same semantics,
    since each engine executes its queue in order."""
    k = 0
    for fn in nc.m.functions:
        for bb in fn.blocks:
            out = []
            for ins in bb.instructions:
                si = ins.sync_info
                waits = list(si.on_wait) if si is not None and si.on_wait else []
                if len(waits) > limit:
                    keep = waits[-limit:]
                    for w in waits[:-limit]:
                        nop = mybir.InstNoOp(
                            name=f"WC-{k}", ins=[], outs=[], engine=ins.engine
                        )
                        k += 1
                        nop.sync_info = mybir.SyncInfo(on_wait=[w], on_update=[])
                        out.append(nop)
                    si.on_wait = keep
                out.append(ins)
            bb.instructions[:] = out
    return k


def build_nc():
    nc = bass.Bass()
    X = nc.declare_dram_parameter("x", [NN_PER_CORE, C, T, VV], F32R, isOutput=False)
    WQK = nc.declare_dram_parameter("wqkT", [C, 2 * C], F32R, isOutput=False)
    WV = nc.declare_dram_parameter("wvT", [C, C], F32R, isOutput=False)
    WP = nc.declare_dram_parameter("wprojT", [C, C], F32R, isOutput=False)
    Y = nc.declare_dram_parameter("y", [NN_PER_CORE, C, T, VV], F32, isOutput=True)

    with tile.TileContext(nc) as tc:
        with (
            tc.tile_pool(name="consts", bufs=1) as consts,
            tc.tile_pool(name="perg", bufs=2) as perg,
            tc.tile_pool(name="pers", bufs=4) as pers,
            tc.tile_pool(name="pbig", bufs=2, space="PSUM") as pbig,
            tc.tile_pool(name="psmall", bufs=1, space="PSUM") as psmall,
        ):
            # ---- load + fp32r-convert the weights (DVE produces every
            # matmul operand so each matmul carries a single wait) ----
            wqk_r, wv_r, wp_r = [], [], []
            for kc in range(4):
                r0 = consts.tile([128, 2 * C], F32R, tag=f"wqkr{kc}")
                nc.sync.dma_start(out=r0, in_=WQK[kc * 128:(kc + 1) * 128, :])
                wqk_r.append(r0)
                r1 = consts.tile([128, C], F32R, tag=f"wvr{kc}")
                nc.sync.dma_start(out=r1, in_=WV[kc * 128:(kc + 1) * 128, :])
                wv_r.append(r1)
                r2 = consts.tile([128, C], F32R, tag=f"wpr{kc}")
                nc.sync.dma_start(out=r2, in_=WP[kc * 128:(kc + 1) * 128, :])
                wp_r.append(r2)

            for g in range(NGROUPS):
                nn = g // (T // TG)
                t0 = (g % (T // TG)) * TG

                # ---- load x slab: 4 c-chunks of [128, 16, 25] ----
                xr, xp = [], []
                for kc in range(4):
                    xc = perg.tile([128, TG, VV], F32R, tag=f"xr{kc}")
                    nc.sync.dma_start(
                        out=xc,
                        in_=X[nn, kc * 128:(kc + 1) * 128, t0:t0 + TG, :],
                    )
                    xr.append(xc)
                    xq = perg.tile([128, TG, 32], F32R, tag=f"xp{kc}")
                    nc.sync.dma_start(
                        out=xq[:, :, 0:VV],
                        in_=X[nn, kc * 128:(kc + 1) * 128, t0:t0 + TG, :],
                    )
                    xp.append(xq)

                # ---- q^T / k^T: out chunk m rows = c' = h*64+d (heads
                # 2m, 2m+1), cols = (b, i);  m 0-3 = q^T, 4-7 = k^T ----
                qkT = []
                for m in range(8):
                    pq = pbig.tile([128, NB], F32, tag="big")
                    for kc in range(4):
                        nc.tensor.matmul(
                            pq[:],
                            wqk_r[kc][:, m * 128:(m + 1) * 128],
                            xr[kc][:].rearrange("p t v -> p (t v)"),
                            start=(kc == 0), stop=(kc == 3),
                        )
                    qc = perg.tile([128, NB], F32, tag=f"qkT{m}")
                    nc.vector.tensor_copy(qc[:], pq[:])
                    qkT.append(qc)

                oT = [perg.tile([128, NB], F32R, tag=f"oT{m}", name=f"oT{m}") for m in range(4)]

                for sub in range(4):
                    bcol0 = sub * 4 * VV

                    pv = pbig.tile([128, C], F32, tag="big", name="pv")
                    for kc in range(4):
                        nc.tensor.matmul(
                            pv[:],
                            xp[kc][:, sub * 4:sub * 4 + 4, :],
                            wv_r[kc][:],
                            start=(kc == 0), stop=(kc == 3),
                        )
                    v2 = [pers.tile([64, C], F32, tag=f"v2{q}", name=f"v2{q}") for q in range(2)]
                    for q in range(2):
                        nc.vector.tensor_copy(v2[q][:, :], pv[q * 64:(q + 1) * 64, :])

                    psm = [
                        psmall.tile([128, 4, VV], F32, tag=f"psm{par}", name=f"psm{par}", bufs=2)
                        for par in range(2)
                    ]
                    for h in range(H):
                        m, par, r0 = h // 2, h % 2, (h % 2) * 64
                        for b4 in range(4):
                            bcol = bcol0 + b4 * VV
                            nc.tensor.matmul(
                                psm[par][b4 * 32:b4 * 32 + 25, m, :],
                                qkT[4 + m][r0:r0 + 64, bcol:bcol + VV],
                                qkT[m][r0:r0 + 64, bcol:bcol + VV],
                                start=True, stop=True,
                                tile_position=(r0, b4 * 32),
                            )

                    e_t = perg.tile([128, VV, H], F32, tag="e_t")
                    for par in range(2):
                        nc.scalar.activation(
                            e_t[:, :, par::2],
                            psm[par][:].rearrange("p m i -> p i m"),
                            mybir.ActivationFunctionType.Exp,
                        )
                    D = perg.tile([128, VV], F32, tag="D")
                    nc.vector.reduce_sum(out=D[:], in_=e_t[:], axis=mybir.AxisListType.X)
                    rD = perg.tile([128, VV], F32, tag="rD")
                    nc.vector.reciprocal(rD[:], D[:])
                    p2 = [pers.tile([64, VV, H], F32, tag=f"p2{q}", name=f"p2{q}") for q in range(2)]
                    for q in range(2):
                        nc.vector.tensor_mul(
                            p2[q][:],
                            e_t[q * 64:(q + 1) * 64, :, :],
                            rD[q * 64:(q + 1) * 64, :]
                            .unsqueeze(2).broadcast_to([64, VV, H]),
                        )

                    po = [
                        psmall.tile([128, 4, 2 * VV], F32, tag=f"po{e}", name=f"po{e}")
                        for e in range(2)
                    ]
                    for b4 in range(4):
                        q, e = b4 // 2, b4 % 2
                        for h in range(H):
                            m, c0 = h // 2, (h % 2) * 64
                            nc.tensor.matmul(
                                po[e][c0:c0 + 64, m, q * VV:(q + 1) * VV],
                                v2[q][e * 32:e * 32 + 25, h * HD:(h + 1) * HD],
                                p2[q][e * 32:e * 32 + 25, :, h],
                                start=True, stop=True,
                                tile_position=(e * 32, c0),
                            )
                    for e in range(2):
                        for m in range(4):
                            dst = oT[m][:].rearrange(
                                "p (b i) -> p b i", i=VV
                            )[:, sub * 4 + e:sub * 4 + e + 3:2, :]
                            nc.vector.tensor_copy(
                                dst, po[e][:, m, :].rearrange(
                                    "p (b i) -> p b i", i=VV
                                )
                            )

                # ---- proj: final^T[co, (b,i)] ----
                for co in range(4):
                    pf = pbig.tile([128, NB], F32, tag="big")
                    for kc in range(4):
                        nc.tensor.matmul(
                            pf[:],
                            wp_r[kc][:, co * 128:(co + 1) * 128],
                            oT[kc][:],
                            start=(kc == 0), stop=(kc == 3),
                        )
                    fin = perg.tile([128, NB], F32, tag=f"fin{co}")
                    nc.vector.tensor_copy(fin[:], pf[:])
                    nc.sync.dma_start(
                        out=Y[nn, co * 128:(co + 1) * 128, t0:t0 + TG, :],
                        in_=fin[:].rearrange("p (t v) -> p t v", t=TG),
                    )
    return nc


LAST_RESULT = {}


def kernel(x: np.ndarray, w_qkv: np.ndarray, w_proj: np.ndarray,
           _trace: bool = False) -> np.ndarray:
    n, c, t, vv = x.shape
    assert (n, c, t, vv) == (16, 512, 256, 25)
    scale = np.float32((c // H) ** -0.5)

    wq = w_qkv[:c] * scale
    wk = w_qkv[c:2 * c]
    wv = w_qkv[2 * c:]
    wqkT = np.ascontiguousarray(np.concatenate([wq, wk], axis=0).T.astype(np.float32))
    wvT = np.ascontiguousarray(wv.T.astype(np.float32))
    wprojT = np.ascontiguousarray(w_proj.T.astype(np.float32))

    nc = build_nc()
    split_excess_waits(nc)
    in_maps = []
    for core in range(N_CORES):
        shard = np.ascontiguousarray(
            x[core * NN_PER_CORE:(core + 1) * NN_PER_CORE].astype(np.float32)
        )
        in_maps.append({"x": shard, "wqkT": wqkT, "wvT": wvT, "wprojT": wprojT})

    kw = {}
    if _trace:
        import tempfile
        kw = dict(trace=True, tmpdir=tempfile.mkdtemp(prefix="attn2_trace_"))
    res = run_bass_kernel_spmd(nc, in_maps, list(range(N_CORES)), **kw)
    LAST_RESULT["res"] = res
    LAST_RESULT["tmpdir"] = kw.get("tmpdir")
    out = np.empty((n, c, t, vv), dtype=np.float32)
    for core in range(N_CORES):
        out[core * NN_PER_CORE:(core + 1) * NN_PER_CORE] = res.results[core]["y"]
    return out


# revision 11
# speedup vs baseline: 1.3590x; 1.3590x over previous
"""Trainium2 Bass kernel for nn_Attention2 (dense transformer block with
softmax over the heads axis).

Computation per (n, t) batch b (B = n*t = 4096 total, X_b = x[n,:,t,:].T is
[vv=25, c=512]):
    qkv = X_b @ w_qkv.T, split into q,k,v heads [h=8, 25, hd=64]
    s[h,i,j] = (q[h,i,:] . k[h,j,:]) / 8      (scale folded into w_q on host)
    p = softmax over h (axis 0)
    o[h,i,:] = sum_j p[h,i,j] v[h,j,:]  -> [25, 512] -> @ w_proj.T
    out[n,:,t,:] = result.T

Sharding: data-parallel over n, 2 n-values (512 batches) per core, 8 cores.

Layout trick: x[n, :, t, :] is naturally X_b^T ([c, vv], c on partitions), so
the qkv and proj matmuls run as weight-stationary batched GEMMs with the
batch dim fused into the moving free dim (16 batches -> N=400).  v is
computed in V[j, c'] layout (j on partitions) via x-slab-stationary matmuls
so the attention-core matmuls need no transposes.  fp32r (1 cycle/row vs 4
for fp32, ~1.6e-4 rel err) is used for the three big GEMMs; the tiny
attention matmuls run fp32 packed onto the PE array with tile_position.
"""
import numpy as np
import concourse.bass as bass
import concourse.mybir as mybir
import concourse.tile as tile
from concourse.bass_utils import run_bass_kernel_spmd
from concourse.vector_clock import ScopedClock, VectorClock

F32 = mybir.dt.float32
F32R = mybir.dt.float32r
F16 = mybir.dt.float16

N_CORES = 8
NN_PER_CORE = 2        # n values per core
T = 256
VV = 25
C = 512
H = 8
HD = 64
TG = 16                # t values (batches) per group
NGROUPS = NN_PER_CORE * (T // TG)   # 32 groups per core
NB = TG * VV           # 400 moving columns per group


def _split_drain_and_barrier(self, tick_clock, wait_clock):
    # walrus caps sync-wait commands at 1 for CTRL_NO; split the kernel-tail
    # drain into one drain per pending proc.
    vc = tick_clock.global_clock
    n = len(vc)
    for i in range(n):
        if vc[i] == 0:
            continue
        sub = VectorClock([vc[j] if j == i else 0 for j in range(n)])
        d = self.nc.sync.drain()
        wait_clock.add_sem_waits(d.ins, ScopedClock({None: sub}))
    self.nc.all_engine_barrier()
    assert self.sems is not None
    popped = self.nc._tile_sem_poison_stack.pop()
    assert popped is self._sem_poison
    self.nc.clear_and_free_semaphores(list(self.sems.allocated().values()))
    self.nc.all_engine_barrier()


tile.TileContext._drain_and_barrier = _split_drain_and_barrier


def split_excess_waits(nc, limit=1):
    """walrus codegen allows very few sync-wait commands per instruction
    (1 for matmul/drain/DMA structs).  Move excess waits onto same-engine
    NoOp carriers inserted just before the instruction — same semantics,
    since each engine executes its queue in order."""
    k = 0
    for fn in nc.m.functions:
        for bb in fn.blocks:
            out = []
            for ins in bb.instructions:
                si = ins.sync_info
                waits = list(si.on_wait) if si is not None and si.on_wait else []
                if len(waits) > limit:
                    keep = waits[-limit:]
                    for w in waits[:-limit]:
                        nop = mybir.InstNoOp(
                            name=f"WC-{k}", ins=[], outs=[], engine=ins.engine
                        )
                        k += 1
                        nop.sync_info = mybir.SyncInfo(on_wait=[w], on_update=[])
                        out.append(nop)
                    si.on_wait = keep
                out.append(ins)
            bb.instructions[:] = out
    return k


def build_nc():
    nc = bass.Bass()
    X = nc.declare_dram_parameter("x", [NN_PER_CORE, C, T, VV], F16, isOutput=False)
    WQK = nc.declare_dram_parameter("wqkT", [C, 2 * C], F16, isOutput=False)
    WV = nc.declare_dram_parameter("wvT", [C, C], F16, isOutput=False)
    WP = nc.declare_dram_parameter("wprojT", [C, C], F16, isOutput=False)
    Y = nc.declare_dram_parameter("y", [NN_PER_CORE, C, T, VV], F32, isOutput=True)

    with tile.TileContext(nc) as tc:
        with (
            tc.tile_pool(name="consts", bufs=1) as consts,
            tc.tile_pool(name="perg", bufs=2) as perg,
            tc.tile_pool(name="pers", bufs=4) as pers,
            tc.tile_pool(name="pbig", bufs=2, space="PSUM") as pbig,
            tc.tile_pool(name="psmall", bufs=1, space="PSUM") as psmall,
        ):
            # ---- load + fp32r-convert the weights (DVE produces every
            # matmul operand so each matmul carries a single wait) ----
            wqk_r, wv_r, wp_r = [], [], []
            for kc in range(4):
                r0 = consts.tile([128, 2 * C], F16, tag=f"wqkr{kc}")
                nc.sync.dma_start(out=r0, in_=WQK[kc * 128:(kc + 1) * 128, :])
                wqk_r.append(r0)
                r1 = consts.tile([128, C], F16, tag=f"wvr{kc}")
                nc.sync.dma_start(out=r1, in_=WV[kc * 128:(kc + 1) * 128, :])
                wv_r.append(r1)
                r2 = consts.tile([128, C], F16, tag=f"wpr{kc}")
                nc.sync.dma_start(out=r2, in_=WP[kc * 128:(kc + 1) * 128, :])
                wp_r.append(r2)

            for g in range(NGROUPS):
                nn = g // (T // TG)
                t0 = (g % (T // TG)) * TG

                # ---- load x slab: 4 c-chunks of [128, 16, 25] ----
                xr, xp = [], []
                for kc in range(4):
                    xc = perg.tile([128, TG, VV], F16, tag=f"xr{kc}")
                    nc.sync.dma_start(
                        out=xc,
                        in_=X[nn, kc * 128:(kc + 1) * 128, t0:t0 + TG, :],
                    )
                    xr.append(xc)
                    xq = perg.tile([128, TG, 32], F16, tag=f"xp{kc}")
                    nc.sync.dma_start(
                        out=xq[:, :, 0:VV],
                        in_=X[nn, kc * 128:(kc + 1) * 128, t0:t0 + TG, :],
                    )
                    xp.append(xq)

                # ---- q^T / k^T: out chunk m rows = c' = h*64+d (heads
                # 2m, 2m+1), cols = (b, i);  m 0-3 = q^T, 4-7 = k^T ----
                qkT = []
                for m in range(8):
                    pq = pbig.tile([128, NB], F32, tag="big")
                    for kc in range(4):
                        nc.tensor.matmul(
                            pq[:],
                            wqk_r[kc][:, m * 128:(m + 1) * 128],
                            xr[kc][:].rearrange("p t v -> p (t v)"),
                            start=(kc == 0), stop=(kc == 3),
                        )
                    qc = perg.tile([128, NB], F16, tag=f"qkT{m}")
                    nc.vector.tensor_copy(qc[:], pq[:])
                    qkT.append(qc)

                oT = [perg.tile([128, NB], F16, tag=f"oT{m}", name=f"oT{m}") for m in range(4)]

                for sub in range(4):
                    bcol0 = sub * 4 * VV

                    pv = pbig.tile([128, C], F32, tag="big", name="pv")
                    for kc in range(4):
                        nc.tensor.matmul(
                            pv[:],
                            xp[kc][:, sub * 4:sub * 4 + 4, :],
                            wv_r[kc][:],
                            start=(kc == 0), stop=(kc == 3),
                        )
                    v2 = [pers.tile([64, C], F16, tag=f"v2{q}", name=f"v2{q}") for q in range(2)]
                    for q in range(2):
                        nc.vector.tensor_copy(v2[q][:, :], pv[q * 64:(q + 1) * 64, :])

                    psm = [
                        psmall.tile([128, 4, VV], F32, tag=f"psm{par}", name=f"psm{par}", bufs=2)
                        for par in range(2)
                    ]
                    for h in range(H):
                        m, par, r0 = h // 2, h % 2, (h % 2) * 64
                        for b4 in range(4):
                            bcol = bcol0 + b4 * VV
                            nc.tensor.matmul(
                                psm[par][b4 * 32:b4 * 32 + 25, m, :],
                                qkT[4 + m][r0:r0 + 64, bcol:bcol + VV],
                                qkT[m][r0:r0 + 64, bcol:bcol + VV],
                                start=True, stop=True,
                                tile_position=(r0, b4 * 32),
                            )

                    e_t = perg.tile([128, VV, H], F32, tag="e_t")
                    for par in range(2):
                        nc.scalar.activation(
                            e_t[:, :, par::2],
                            psm[par][:].rearrange("p m i -> p i m"),
                            mybir.ActivationFunctionType.Exp,
                        )
                    D = perg.tile([128, VV], F32, tag="D")
                    nc.vector.reduce_sum(out=D[:], in_=e_t[:], axis=mybir.AxisListType.X)
                    rD = perg.tile([128, VV], F32, tag="rD")
                    nc.vector.reciprocal(rD[:], D[:])
                    p2 = [pers.tile([64, VV, H], F16, tag=f"p2{q}", name=f"p2{q}") for q in range(2)]
                    for q in range(2):
                        nc.vector.tensor_mul(
                            p2[q][:],
                            e_t[q * 64:(q + 1) * 64, :, :],
                            rD[q * 64:(q + 1) * 64, :]
                            .unsqueeze(2).broadcast_to([64, VV, H]),
                        )

                    po = [
                        psmall.tile([128, 4, 2 * VV], F32, tag=f"po{e}", name=f"po{e}")
                        for e in range(2)
                    ]
                    for b4 in range(4):
                        q, e = b4 // 2, b4 % 2
                        for h in range(H):
                            m, c0 = h // 2, (h % 2) * 64
                            nc.tensor.matmul(
                                po[e][c0:c0 + 64, m, q * VV:(q + 1) * VV],
                                v2[q][e * 32:e * 32 + 25, h * HD:(h + 1) * HD],
                                p2[q][e * 32:e * 32 + 25, :, h],
                                start=True, stop=True,
                                tile_position=(e * 32, c0),
                            )
                    for e in range(2):
                        for m in range(4):
                            dst = oT[m][:].rearrange(
                                "p (b i) -> p b i", i=VV
                            )[:, sub * 4 + e:sub * 4 + e + 3:2, :]
                            nc.vector.tensor_copy(
                                dst, po[e][:, m, :].rearrange(
                                    "p (b i) -> p b i", i=VV
                                )
                            )

                # ---- proj: final^T[co, (b,i)] ----
                for co in range(4):
                    pf = pbig.tile([128, NB], F32, tag="big")
                    for kc in range(4):
                        nc.tensor.matmul(
                            pf[:],
                            wp_r[kc][:, co * 128:(co + 1) * 128],
                            oT[kc][:],
                            start=(kc == 0), stop=(kc == 3),
                        )
                    fin = perg.tile([128, NB], F32, tag=f"fin{co}")
                    nc.vector.tensor_copy(fin[:], pf[:])
                    nc.sync.dma_start(
                        out=Y[nn, co * 128:(co + 1) * 128, t0:t0 + TG, :],
                        in_=fin[:].rearrange("p (t v) -> p t v", t=TG),
                    )
    return nc


LAST_RESULT = {}


def kernel(x: np.ndarray, w_qkv: np.ndarray, w_proj: np.ndarray,
           _trace: bool = False) -> np.ndarray:
    n, c, t, vv = x.shape
    assert (n, c, t, vv) == (16, 512, 256, 25)
    scale = np.float32((c // H) ** -0.5)

    wq = w_qkv[:c] * scale
    wk = w_qkv[c:2 * c]
    wv = w_qkv[2 * c:]
    wqkT = np.ascontiguousarray(np.concatenate([wq, wk], axis=0).T.astype(np.float16))
    wvT = np.ascontiguousarray(wv.T.astype(np.float16))
    wprojT = np.ascontiguousarray(w_proj.T.astype(np.float16))

    nc = build_nc()
    split_excess_waits(nc)
    in_maps = []
    for core in range(N_CORES):
        shard = np.ascontiguousarray(
            x[core * NN_PER_CORE:(core + 1) * NN_PER_CORE].astype(np.float16)
        )
        in_maps.append({"x": shard, "wqkT": wqkT, "wvT": wvT, "wprojT": wprojT})

    kw = {}
    if _trace:
        import tempfile
        kw = dict(trace=True, tmpdir=tempfile.mkdtemp(prefix="attn2_trace_"))
    res = run_bass_kernel_spmd(nc, in_maps, list(range(N_CORES)), **kw)
    LAST_RESULT["res"] = res
    LAST_RESULT["tmpdir"] = kw.get("tmpdir")
    out = np.empty((n, c, t, vv), dtype=np.float32)
    for core in range(N_CORES):
        out[core * NN_PER_CORE:(core + 1) * NN_PER_CORE] = res.results[core]["y"]
    return out


# revision 12
# speedup vs baseline: 1.4532x; 1.0693x over previous
"""Trainium2 Bass kernel for nn_Attention2 (dense transformer block with
softmax over the heads axis).

Computation per (n, t) batch b (B = n*t = 4096 total, X_b = x[n,:,t,:].T is
[vv=25, c=512]):
    qkv = X_b @ w_qkv.T, split into q,k,v heads [h=8, 25, hd=64]
    s[h,i,j] = (q[h,i,:] . k[h,j,:]) / 8      (scale folded into w_q on host)
    p = softmax over h (axis 0)
    o[h,i,:] = sum_j p[h,i,j] v[h,j,:]  -> [25, 512] -> @ w_proj.T
    out[n,:,t,:] = result.T

Sharding: data-parallel over n, 2 n-values (512 batches) per core, 8 cores.

Layout trick: x[n, :, t, :] is naturally X_b^T ([c, vv], c on partitions), so
the qkv and proj matmuls run as weight-stationary batched GEMMs with the
batch dim fused into the moving free dim (16 batches -> N=400).  v is
computed in V[j, c'] layout (j on partitions) via x-slab-stationary matmuls
so the attention-core matmuls need no transposes.  fp32r (1 cycle/row vs 4
for fp32, ~1.6e-4 rel err) is used for the three big GEMMs; the tiny
attention matmuls run fp32 packed onto the PE array with tile_position.
"""
import numpy as np
import concourse.bass as bass
import concourse.mybir as mybir
import concourse.tile as tile
from concourse.bass_utils import run_bass_kernel_spmd
from concourse.vector_clock import ScopedClock, VectorClock

F32 = mybir.dt.float32
F32R = mybir.dt.float32r
F16 = mybir.dt.float16

N_CORES = 8
NN_PER_CORE = 2        # n values per core
T = 256
VV = 25
C = 512
H = 8
HD = 64
TG = 16                # t values (batches) per group
NGROUPS = NN_PER_CORE * (T // TG)   # 32 groups per core
NB = TG * VV           # 400 moving columns per group


def _split_drain_and_barrier(self, tick_clock, wait_clock):
    # walrus caps sync-wait commands at 1 for CTRL_NO; split the kernel-tail
    # drain into one drain per pending proc.
    vc = tick_clock.global_clock
    n = len(vc)
    for i in range(n):
        if vc[i] == 0:
            continue
        sub = VectorClock([vc[j] if j == i else 0 for j in range(n)])
        d = self.nc.sync.drain()
        wait_clock.add_sem_waits(d.ins, ScopedClock({None: sub}))
    self.nc.all_engine_barrier()
    assert self.sems is not None
    popped = self.nc._tile_sem_poison_stack.pop()
    assert popped is self._sem_poison
    self.nc.clear_and_free_semaphores(list(self.sems.allocated().values()))
    self.nc.all_engine_barrier()


tile.TileContext._drain_and_barrier = _split_drain_and_barrier


def split_excess_waits(nc, limit=1):
    """walrus codegen allows very few sync-wait commands per instruction
    (1 for matmul/drain/DMA structs).  Move excess waits onto same-engine
    NoOp carriers inserted just before the instruction — same semantics,
    since each engine executes its queue in order."""
    k = 0
    for fn in nc.m.functions:
        for bb in fn.blocks:
            out = []
            for ins in bb.instructions:
                si = ins.sync_info
                waits = list(si.on_wait) if si is not None and si.on_wait else []
                if len(waits) > limit:
                    keep = waits[-limit:]
                    for w in waits[:-limit]:
                        nop = mybir.InstNoOp(
                            name=f"WC-{k}", ins=[], outs=[], engine=ins.engine
                        )
                        k += 1
                        nop.sync_info = mybir.SyncInfo(on_wait=[w], on_update=[])
                        out.append(nop)
                    si.on_wait = keep
                out.append(ins)
            bb.instructions[:] = out
    return k


def build_nc():
    nc = bass.Bass()
    X = nc.declare_dram_parameter("x", [NN_PER_CORE, C, T, VV], F16, isOutput=False)
    WQK = nc.declare_dram_parameter("wqkT", [C, 2 * C], F16, isOutput=False)
    WV = nc.declare_dram_parameter("wvT", [C, C], F16, isOutput=False)
    WP = nc.declare_dram_parameter("wprojT", [C, C], F16, isOutput=False)
    Y = nc.declare_dram_parameter("y", [NN_PER_CORE, C, T, VV], F32, isOutput=True)

    with tile.TileContext(nc) as tc:
        with (
            tc.tile_pool(name="consts", bufs=1) as consts,
            tc.tile_pool(name="perg", bufs=2) as perg,
            tc.tile_pool(name="pers", bufs=4) as pers,
            tc.tile_pool(name="pbig", bufs=2, space="PSUM") as pbig,
            tc.tile_pool(name="psmall", bufs=1, space="PSUM") as psmall,
        ):
            # ---- load + fp32r-convert the weights (DVE produces every
            # matmul operand so each matmul carries a single wait) ----
            wqk_r, wv_r, wp_r = [], [], []
            for kc in range(4):
                r0 = consts.tile([128, 2 * C], F16, tag=f"wqkr{kc}")
                nc.sync.dma_start(out=r0, in_=WQK[kc * 128:(kc + 1) * 128, :])
                wqk_r.append(r0)
                r1 = consts.tile([128, C], F16, tag=f"wvr{kc}")
                nc.sync.dma_start(out=r1, in_=WV[kc * 128:(kc + 1) * 128, :])
                wv_r.append(r1)
                r2 = consts.tile([128, C], F16, tag=f"wpr{kc}")
                nc.sync.dma_start(out=r2, in_=WP[kc * 128:(kc + 1) * 128, :])
                wp_r.append(r2)

            for g in range(NGROUPS):
                nn = g // (T // TG)
                t0 = (g % (T // TG)) * TG

                # ---- load x slab: 4 c-chunks of [128, 16, 25] ----
                xr, xp = [], []
                for kc in range(4):
                    xc = perg.tile([128, TG, VV], F16, tag=f"xr{kc}")
                    nc.sync.dma_start(
                        out=xc,
                        in_=X[nn, kc * 128:(kc + 1) * 128, t0:t0 + TG, :],
                    )
                    xr.append(xc)
                    xq = perg.tile([128, TG, 32], F16, tag=f"xp{kc}")
                    nc.sync.dma_start(
                        out=xq[:, :, 0:VV],
                        in_=X[nn, kc * 128:(kc + 1) * 128, t0:t0 + TG, :],
                    )
                    xp.append(xq)

                # ---- q^T / k^T: out chunk m rows = c' = h*64+d (heads
                # 2m, 2m+1), cols = (b, i);  m 0-3 = q^T, 4-7 = k^T ----
                qkT = []
                for m in range(8):
                    pq = pbig.tile([128, NB], F32, tag="big")
                    for kc in range(4):
                        nc.tensor.matmul(
                            pq[:],
                            wqk_r[kc][:, m * 128:(m + 1) * 128],
                            xr[kc][:].rearrange("p t v -> p (t v)"),
                            start=(kc == 0), stop=(kc == 3),
                        )
                    qc = perg.tile([128, NB], F16, tag=f"qkT{m}")
                    nc.vector.tensor_copy(qc[:, 0:NB // 2], pq[:, 0:NB // 2])
                    nc.vector.tensor_copy(qc[:, NB // 2:], pq[:, NB // 2:])
                    qkT.append(qc)

                oT = [perg.tile([128, NB], F16, tag=f"oT{m}", name=f"oT{m}") for m in range(4)]

                for sub in range(4):
                    bcol0 = sub * 4 * VV

                    pv = pbig.tile([128, C], F32, tag="big", name="pv")
                    for kc in range(4):
                        nc.tensor.matmul(
                            pv[:],
                            xp[kc][:, sub * 4:sub * 4 + 4, :],
                            wv_r[kc][:],
                            start=(kc == 0), stop=(kc == 3),
                        )
                    v2 = [pers.tile([64, C], F16, tag=f"v2{q}", name=f"v2{q}") for q in range(2)]
                    for q in range(2):
                        nc.scalar.activation(
                            v2[q][:, :], pv[q * 64:(q + 1) * 64, :],
                            mybir.ActivationFunctionType.Copy,
                        )

                    psm = [
                        psmall.tile([128, 4, VV], F32, tag=f"psm{par}", name=f"psm{par}", bufs=2)
                        for par in range(2)
                    ]
                    for h in range(H):
                        m, par, r0 = h // 2, h % 2, (h % 2) * 64
                        for b4 in range(4):
                            bcol = bcol0 + b4 * VV
                            nc.tensor.matmul(
                                psm[par][b4 * 32:b4 * 32 + 25, m, :],
                                qkT[4 + m][r0:r0 + 64, bcol:bcol + VV],
                                qkT[m][r0:r0 + 64, bcol:bcol + VV],
                                start=True, stop=True,
                                tile_position=(r0, b4 * 32),
                            )

                    e_t = perg.tile([128, VV, H], F32, tag="e_t")
                    for par in range(2):
                        nc.scalar.activation(
                            e_t[:, :, par::2],
                            psm[par][:].rearrange("p m i -> p i m"),
                            mybir.ActivationFunctionType.Exp,
                        )
                    D = perg.tile([128, VV], F32, tag="D")
                    nc.vector.reduce_sum(out=D[:], in_=e_t[:], axis=mybir.AxisListType.X)
                    rD = perg.tile([128, VV], F32, tag="rD")
                    nc.vector.reciprocal(rD[:], D[:])
                    p2 = [pers.tile([64, VV, H], F16, tag=f"p2{q}", name=f"p2{q}") for q in range(2)]
                    for q in range(2):
                        nc.vector.tensor_mul(
                            p2[q][:],
                            e_t[q * 64:(q + 1) * 64, :, :],
                            rD[q * 64:(q + 1) * 64, :]
                            .unsqueeze(2).broadcast_to([64, VV, H]),
                        )

                    po = [
                        psmall.tile([128, 4, 2 * VV], F32, tag=f"po{e}", name=f"po{e}")
                        for e in range(2)
                    ]
                    for b4 in range(4):
                        q, e = b4 // 2, b4 % 2
                        for h in range(H):
                            m, c0 = h // 2, (h % 2) * 64
                            nc.tensor.matmul(
                                po[e][c0:c0 + 64, m, q * VV:(q + 1) * VV],
                                v2[q][e * 32:e * 32 + 25, h * HD:(h + 1) * HD],
                                p2[q][e * 32:e * 32 + 25, :, h],
                                start=True, stop=True,
                                tile_position=(e * 32, c0),
                            )
                    for e in range(2):
                        for m in range(4):
                            dst = oT[m][:].rearrange(
                                "p (b i) -> p b i", i=VV
                            )[:, sub * 4 + e:sub * 4 + e + 3:2, :]
                            nc.vector.tensor_copy(
                                dst, po[e][:, m, :].rearrange(
                                    "p (b i) -> p b i", i=VV
                                )
                            )

                # ---- proj: final^T[co, (b,i)] ----
                for co in range(4):
                    pf = pbig.tile([128, NB], F32, tag="big")
                    for kc in range(4):
                        nc.tensor.matmul(
                            pf[:],
                            wp_r[kc][:, co * 128:(co + 1) * 128],
                            oT[kc][:],
                            start=(kc == 0), stop=(kc == 3),
                        )
                    fin = perg.tile([128, NB], F32, tag=f"fin{co}")
                    nc.scalar.activation(
                        fin[:], pf[:], mybir.ActivationFunctionType.Copy,
                    )
                    nc.sync.dma_start(
                        out=Y[nn, co * 128:(co + 1) * 128, t0:t0 + TG, :],
                        in_=fin[:].rearrange("p (t v) -> p t v", t=TG),
                    )
    return nc


LAST_RESULT = {}


def kernel(x: np.ndarray, w_qkv: np.ndarray, w_proj: np.ndarray,
           _trace: bool = False) -> np.ndarray:
    n, c, t, vv = x.shape
    assert (n, c, t, vv) == (16, 512, 256, 25)
    scale = np.float32((c // H) ** -0.5)

    wq = w_qkv[:c] * scale
    wk = w_qkv[c:2 * c]
    wv = w_qkv[2 * c:]
    wqkT = np.ascontiguousarray(np.concatenate([wq, wk], axis=0).T.astype(np.float16))
    wvT = np.ascontiguousarray(wv.T.astype(np.float16))
    wprojT = np.ascontiguousarray(w_proj.T.astype(np.float16))

    nc = build_nc()
    split_excess_waits(nc)
    in_maps = []
    for core in range(N_CORES):
        shard = np.ascontiguousarray(
            x[core * NN_PER_CORE:(core + 1) * NN_PER_CORE].astype(np.float16)
        )
        in_maps.append({"x": shard, "wqkT": wqkT, "wvT": wvT, "wprojT": wprojT})

    kw = {}
    if _trace:
        import tempfile
        kw = dict(trace=True, tmpdir=tempfile.mkdtemp(prefix="attn2_trace_"))
    res = run_bass_kernel_spmd(nc, in_maps, list(range(N_CORES)), **kw)
    LAST_RESULT["res"] = res
    LAST_RESULT["tmpdir"] = kw.get("tmpdir")
    out = np.empty((n, c, t, vv), dtype=np.float32)
    for core in range(N_CORES):
        out[core * NN_PER_CORE:(core + 1) * NN_PER_CORE] = res.results[core]["y"]
    return out
